# revision 15
# baseline (speedup 1.0000x reference)
"""MoE router kernel for Trainium2 (Bass/Tile), SPMD across 8 NeuronCores.

Problem: nn_MoERouter (B=8, T=4096, D=1024, E=64, TOP_K=2).

  router_logits = (x * mask) @ W.T * mask          # (B, T, E)
  router_probs  = softmax(router_logits) * mask
  expert_weights, expert_indices = top_k(probs, 2), renormalized, masked
  padded tokens get index -1

Sharding: data-parallel over the batch axis; core b handles x[b] (4096
tokens). W is tiny and replicated. No cross-core communication.

Matmul strategy (per core): plain fp32 matmul on TRN2 lowers to 2
half-rate passes (~8 ns/output-col measured) and float32r is only
~tf32-accurate (1.5e-4 — would flip near-tie expert indices). Instead we
use an error-compensated fp16 split computed on the host:

  x = xh + xls/2048,  W = Wh + Wls/2048   (xh/xls/Wh/Wls all fp16)
  logits = xh@Wh + (xh@Wls + xls@Wh)/2048    (drops xls@Wls ~ 2^-22)

Max logits error ~4e-6 (verified on the real inputs: 0/65536 index
flips), DMA volume unchanged (2+2 bytes/elem), and the matmuls run at
full 1 cycle/row rate.

Layouts: the contraction (d) must be on SBUF partitions and DMA
descriptors want long contiguous per-partition runs, so the host
pre-tiles x into the exact per-block SBUF layout
[n_blk, 128p, 8chunk, 512tok] (every DMA is a fully contiguous 1 MB
read, 8 KB per partition). logits/probs are likewise written in packed
per-block layout [n_blk, 128p, 4sub, 64e] (1 KB/partition runs) and
unpacked on the host.

Compute: W-stationary matmuls, N=512 tokens, out = logits.T [64, 512]
in PSUM. The main term accumulates in PSUM partitions 0:64 and both
correction terms in 64:128 (concurrent column-packed PE tiles), then
ACT moves the main half to SBUF and one DVE scalar_tensor_tensor adds
correction/2048 (DVE has a single PSUM read port). Four PE transposes
per block bring logits back to [128 tok, 64 exp], ACT does a batched
exp, DVE does softmax scaling + top-8 with indices
(InstMax/InstMaxIndex match jax.lax.top_k tie order).
"""

import os
import sys

import numpy as np

for _p in ("/opt/trn_rl_repo", "/opt/pypackages"):
    if _p not in sys.path and os.path.isdir(_p):
        sys.path.append(_p)

import concourse.bass as bass
import concourse.mybir as mybir
from concourse import bacc
from concourse.masks import make_identity
from concourse.tile import TileContext

F32 = mybir.dt.float32
F16 = mybir.dt.float16
I32 = mybir.dt.int32
U32 = mybir.dt.uint32

B, T, D, E, TOP_K = 8, 4096, 1024, 64, 2
N_CORES = 8
P = 128                    # SBUF partitions
D_CHUNKS = D // P          # 8 contraction chunks
TOK_BLK = 512              # tokens per block (matmul free dim)
SUBS = TOK_BLK // P        # 4 token tiles per block
SPLIT_SCALE = 2048.0       # 2^11 residual scale for the fp16 split


def _bcast(ap: bass.AP, n: int) -> bass.AP:
    """Append a step-0 dim of size n (free-dim broadcast for DVE reads)."""
    return bass.AP(tensor=ap.tensor, offset=ap.offset, ap=[*ap.ap, [0, n]])


def build_moe_router(t_core: int = T) -> bacc.Bacc:
    """Build the per-core Bass program. t_core tokens per core (mult of 512)."""
    assert t_core % TOK_BLK == 0
    n_blk = t_core // TOK_BLK
    n_tiles = t_core // P

    nc = bacc.Bacc("TRN2", target_bir_lowering=False, debug=False)

    xP = nc.dram_tensor("xP", [n_blk, P, 2, D_CHUNKS, TOK_BLK], F16, kind="ExternalInput")
    whT = nc.dram_tensor("whT", [D, E], F16, kind="ExternalInput")
    wlT = nc.dram_tensor("wlT", [D, E], F16, kind="ExternalInput")
    maskf = nc.dram_tensor("maskf", [P, n_tiles], F32, kind="ExternalInput")
    logits_d = nc.dram_tensor("logits", [n_blk, P, SUBS, E], F32, kind="ExternalOutput")
    probs_d = nc.dram_tensor("probs", [n_blk, P, SUBS, E], F32, kind="ExternalOutput")
    weights_d = nc.dram_tensor("weights", [P, n_tiles, TOP_K], F32, kind="ExternalOutput")
    indices_d = nc.dram_tensor("indices", [P, n_tiles, TOP_K], I32, kind="ExternalOutput")

    whT_t = whT.rearrange("(c p) e -> p c e", p=P)        # [128, 8, 64]
    wlT_t = wlT.rearrange("(c p) e -> p c e", p=P)

    MUL = mybir.AluOpType.mult
    ADD = mybir.AluOpType.add

    with TileContext(nc) as tc:
        with (
            tc.tile_pool(name="xpool", bufs=4) as xpool,
            tc.tile_pool(name="consts", bufs=1) as consts,
            tc.tile_pool(name="psT", bufs=3, space="PSUM") as psT,
            tc.tile_pool(name="psL", bufs=2, space="PSUM") as psL,
            tc.tile_pool(name="psink", bufs=1, space="PSUM") as psink,
            tc.tile_pool(name="stage", bufs=3) as stage,
            tc.tile_pool(name="small", bufs=6) as small,
            tc.tile_pool(name="accs", bufs=1) as accs,
        ):
            wh_sb = consts.tile([P, D_CHUNKS, E], F16)
            wl_sb = consts.tile([P, D_CHUNKS, E], F16)
            nc.sync.dma_start(out=wh_sb, in_=whT_t)
            nc.sync.dma_start(out=wl_sb, in_=wlT_t)
            maskf_sb = consts.tile([P, n_tiles], F32)
            nc.sync.dma_start(out=maskf_sb, in_=maskf[:, :])
            ident = consts.tile([E, E], F32)
            make_identity(nc, ident)

            top8 = accs.tile([P, n_tiles, 8], F32)
            idx8 = accs.tile([P, n_tiles, 8], U32)

            # PE sink matmuls absorb DMA-completion waits so real matmuls
            # carry at most one wait (walrus limit on Matmult sync waits).
            sink_ps = psink.tile([1, 1], F32)
            nc.tensor.matmul(sink_ps, lhsT=wh_sb[:, 0, 0:1], rhs=wh_sb[:, 0, 0:1])
            nc.tensor.matmul(sink_ps, lhsT=wl_sb[:, 0, 0:1], rhs=wl_sb[:, 0, 0:1])

            # HAM warmup: keep the PE busy ~4us so real matmuls start at
            # 2.4 GHz instead of 1.2 (results discarded).
            warm_ps = psink.tile([E, TOK_BLK], F32)
            warm_rhs = bass.AP(
                tensor=wh_sb.tensor, offset=wh_sb[:, 0, 0:1].offset,
                ap=[wh_sb[:, 0, 0:1].ap[0], [0, TOK_BLK]],
            )
            for w in range(16):
                nc.tensor.matmul(
                    warm_ps, lhsT=wh_sb[:, w % D_CHUNKS, :], rhs=warm_rhs,
                    start=(w == 0), stop=(w == 15), skip_group_check=True,
                )

            for blk in range(n_blk):
                x_sb = xpool.tile([P, 2, D_CHUNKS, TOK_BLK], F16)
                nc.sync.dma_start(out=x_sb, in_=xP[blk, :, :, :, :])
                xh_sb = x_sb[:, 0]
                xl_sb = x_sb[:, 1]
                nc.tensor.matmul(sink_ps, lhsT=x_sb[:, 0, 0, 0:1], rhs=x_sb[:, 0, 0, 0:1])

                # logits.T: main term -> PSUM partitions 0:64, correction
                # terms (x2048) -> 64:128; the two column tiles run
                # concurrently on the PE array.
                lgT_ps = psT.tile([P, TOK_BLK], F32)
                for c in range(D_CHUNKS):
                    nc.tensor.matmul(
                        lgT_ps[0:E, :], lhsT=wh_sb[:, c, :], rhs=xh_sb[:, c, :],
                        start=(c == 0), stop=(c == D_CHUNKS - 1),
                        skip_group_check=True,
                    )
                    nc.tensor.matmul(
                        lgT_ps[E : 2 * E, :], lhsT=wl_sb[:, c, :], rhs=xh_sb[:, c, :],
                        start=(c == 0), stop=False, skip_group_check=True,
                    )
                    nc.tensor.matmul(
                        lgT_ps[E : 2 * E, :], lhsT=wh_sb[:, c, :], rhs=xl_sb[:, c, :],
                        start=False, stop=(c == D_CHUNKS - 1), skip_group_check=True,
                    )

                # lgT = correction/2048 + main   [64, 512] fp32 in SBUF
                # (DVE has one PSUM read port: ACT moves the main half to
                # SBUF, DVE adds the scaled correction from PSUM onto it.)
                lgT_sb = stage.tile([E, TOK_BLK], F32)
                nc.scalar.copy(lgT_sb, lgT_ps[0:E, :])
                nc.vector.scalar_tensor_tensor(
                    out=lgT_sb, in0=lgT_ps[E : 2 * E, :], scalar=1.0 / SPLIT_SCALE,
                    in1=lgT_sb, op0=MUL, op1=ADD,
                )

                # transpose back to [128 tok, 64 exp] tiles (PSUM, one bank)
                lg_ps = psL.tile([P, SUBS, E], F32)
                for sub in range(SUBS):
                    nc.tensor.matmul(
                        lg_ps[:, sub, :], lhsT=lgT_sb[:, sub * P : (sub + 1) * P],
                        rhs=ident, is_transpose=True, skip_group_check=True,
                    )

                mask_blk = maskf_sb[:, blk * SUBS : (blk + 1) * SUBS]  # [128, 4]

                # masked logits PSUM -> SBUF (also the DMA staging buffer)
                lg_sb = stage.tile([P, SUBS, E], F32)
                nc.vector.tensor_mul(lg_sb, lg_ps, _bcast(mask_blk, E))
                nc.scalar.dma_start(out=logits_d[blk, :, :, :], in_=lg_sb)

                # exps (unmasked is fine: masked rows are overridden later)
                exp_sb = stage.tile([P, SUBS, E], F32)
                nc.scalar.activation(
                    out=exp_sb, in_=lg_ps, func=mybir.ActivationFunctionType.Exp
                )

                sums = small.tile([P, SUBS, 1], F32)
                nc.vector.reduce_sum(sums, exp_sb, axis=mybir.AxisListType.X)
                r_t = small.tile([P, SUBS], F32)
                nc.vector.reciprocal(r_t, sums[:, :, 0])
                r2_t = small.tile([P, SUBS], F32)
                nc.vector.tensor_mul(r2_t, r_t, mask_blk)
                pr_sb = stage.tile([P, SUBS, E], F32)
                nc.gpsimd.tensor_mul(pr_sb, exp_sb, _bcast(r2_t[:, :], E))
                nc.scalar.dma_start(out=probs_d[blk, :, :, :], in_=pr_sb)

                for sub in range(SUBS):
                    i = blk * SUBS + sub
                    nc.vector.max(out=top8[:, i, :], in_=exp_sb[:, sub, :])
                    nc.vector.max_index(
                        out=idx8[:, i, :], in_max=top8[:, i, :],
                        in_values=exp_sb[:, sub, :],
                    )

            # ---- tail: renormalized top-2 weights + masked indices ----
            s_t = accs.tile([P, n_tiles], F32)
            nc.gpsimd.tensor_add(s_t, top8[:, :, 0], top8[:, :, 1])
            rs_t = accs.tile([P, n_tiles], F32)
            nc.vector.reciprocal(rs_t, s_t)
            nc.vector.tensor_mul(rs_t, rs_t, maskf_sb)
            w_out = accs.tile([P, n_tiles, TOP_K], F32)
            for k in range(TOP_K):
                nc.vector.tensor_mul(w_out[:, :, k], top8[:, :, k], rs_t)
            nc.scalar.dma_start(out=weights_d[:, :, :], in_=w_out)

            # indices: (idx + 1) * mask - 1  (exact in fp32)
            idxf = accs.tile([P, n_tiles, TOP_K], F32)
            nc.gpsimd.tensor_copy(idxf, idx8[:, :, 0:TOP_K])
            for k in range(TOP_K):
                nc.gpsimd.tensor_scalar_add(idxf[:, :, k], idxf[:, :, k], 1.0)
                nc.gpsimd.tensor_mul(idxf[:, :, k], idxf[:, :, k], maskf_sb)
                nc.gpsimd.tensor_scalar_add(idxf[:, :, k], idxf[:, :, k], -1.0)
            idxi = accs.tile([P, n_tiles, TOP_K], I32)
            nc.gpsimd.tensor_copy(idxi, idxf)
            nc.scalar.dma_start(out=indices_d[:, :, :], in_=idxi)

    # Legalization (splits >1-wait instructions into event-semaphore ops,
    # moves matmul waits to ldweights) — required by walrus codegen.
    nc.compile()
    return nc


_NC_CACHE: dict[int, bacc.Bacc] = {}


def _get_nc(t_core: int = T) -> bacc.Bacc:
    if t_core not in _NC_CACHE:
        _NC_CACHE[t_core] = build_moe_router(t_core)
    return _NC_CACHE[t_core]


def _split16(a: np.ndarray):
    hi = a.astype(np.float16)
    lo = ((a - hi.astype(np.float32)) * SPLIT_SCALE).astype(np.float16)
    return hi, lo


def _pack_x(xh: np.ndarray, xl: np.ndarray, t_core: int) -> np.ndarray:
    """2x [T, D] fp16 -> [n_blk, 128p, 2, 8c, 512t] matching the SBUF tiles."""
    n_blk = t_core // TOK_BLK
    both = np.stack([xh, xl], axis=0)  # [2, T, D]
    return np.ascontiguousarray(
        both.reshape(2, n_blk, TOK_BLK, D_CHUNKS, P).transpose(1, 4, 0, 3, 2)
    )


def make_in_maps(x: np.ndarray, x_mask: np.ndarray, W: np.ndarray):
    """Shard full inputs into per-core input maps (host-side layout prep)."""
    t_core = x.shape[1]
    n_tiles = t_core // P
    wh, wl = _split16(np.asarray(W, dtype=np.float32))
    whT = np.ascontiguousarray(wh.T)
    wlT = np.ascontiguousarray(wl.T)
    in_maps = []
    for b in range(x.shape[0]):
        xh, xl = _split16(np.asarray(x[b], dtype=np.float32))
        mf = np.ascontiguousarray(
            np.asarray(x_mask[b], dtype=np.float32).reshape(n_tiles, P).T
        )
        in_maps.append(
            {
                "xP": _pack_x(xh, xl, t_core),
                "whT": whT,
                "wlT": wlT,
                "maskf": mf,
            }
        )
    return in_maps


def _unpack_te(a: np.ndarray, t_core: int) -> np.ndarray:
    """[n_blk, 128p, 4sub, E] -> [T, E]."""
    return np.ascontiguousarray(
        a.transpose(0, 2, 1, 3).reshape(t_core, a.shape[-1])
    )


def _unpack_tk(a: np.ndarray, t_core: int) -> np.ndarray:
    """[128p, n_tiles, K] -> [T, K]."""
    return np.ascontiguousarray(a.transpose(1, 0, 2).reshape(t_core, a.shape[-1]))


def run_kernel(x, x_mask, W, trace: bool = False, trace_kwargs: dict | None = None):
    """Run on hardware; returns (outputs_tuple, BassKernelResults)."""
    from concourse.bass_utils import run_bass_kernel_spmd

    x = np.asarray(x)
    x_mask = np.asarray(x_mask)
    W = np.asarray(W)
    n_cores, t_core = x.shape[0], x.shape[1]
    nc = _get_nc(t_core)
    in_maps = make_in_maps(x, x_mask, W)
    res = run_bass_kernel_spmd(
        nc,
        in_maps,
        core_ids=list(range(n_cores)),
        trace=trace,
        **(trace_kwargs or {}),
    )
    ew = np.stack([_unpack_tk(res.results[b]["weights"], t_core) for b in range(n_cores)])
    ei = np.stack([_unpack_tk(res.results[b]["indices"], t_core) for b in range(n_cores)])
    rl = np.stack([_unpack_te(res.results[b]["logits"], t_core) for b in range(n_cores)])
    rp = np.stack([_unpack_te(res.results[b]["probs"], t_core) for b in range(n_cores)])
    return (ew, ei, rl, rp), res


def kernel(**inputs):
    outs, _ = run_kernel(
        inputs["x"], inputs["x_mask"], inputs["W"],
        trace=os.environ.get("MOE_TRACE", "") == "1",
    )
    return outs


# revision 16
# speedup vs baseline: 1.0323x; 1.0323x over previous
"""MoE router kernel for Trainium2 (Bass/Tile), SPMD across 8 NeuronCores.

Problem: nn_MoERouter (B=8, T=4096, D=1024, E=64, TOP_K=2).

  router_logits = (x * mask) @ W.T * mask          # (B, T, E)
  router_probs  = softmax(router_logits) * mask
  expert_weights, expert_indices = top_k(probs, 2), renormalized, masked
  padded tokens get index -1

Sharding: data-parallel over the batch axis; core b handles x[b] (4096
tokens). W is tiny and replicated. No cross-core communication.

Matmul strategy (per core): plain fp32 matmul on TRN2 lowers to 2
half-rate passes (~8 ns/output-col measured) and float32r is only
~tf32-accurate (1.5e-4 — would flip near-tie expert indices). Instead we
use an error-compensated fp16 split computed on the host:

  x = xh + xls/2048,  W = Wh + Wls/2048   (xh/xls/Wh/Wls all fp16)
  logits = xh@Wh + (xh@Wls + xls@Wh)/2048    (drops xls@Wls ~ 2^-22)

Max logits error ~4e-6 (verified on the real inputs: 0/65536 index
flips), DMA volume unchanged (2+2 bytes/elem), and the matmuls run at
full 1 cycle/row rate.

Layouts: the contraction (d) must be on SBUF partitions and DMA
descriptors want long contiguous per-partition runs, so the host
pre-tiles x into the exact per-block SBUF layout
[n_blk, 128p, 8chunk, 512tok] (every DMA is a fully contiguous 1 MB
read, 8 KB per partition). logits/probs are likewise written in packed
per-block layout [n_blk, 128p, 4sub, 64e] (1 KB/partition runs) and
unpacked on the host.

Compute: W-stationary matmuls, N=512 tokens, out = logits.T [64, 512]
in PSUM. The main term accumulates in PSUM partitions 0:64 and both
correction terms in 64:128 (concurrent column-packed PE tiles), then
ACT moves the main half to SBUF and one DVE scalar_tensor_tensor adds
correction/2048 (DVE has a single PSUM read port). Four PE transposes
per block bring logits back to [128 tok, 64 exp], ACT does a batched
exp, DVE does softmax scaling + top-8 with indices
(InstMax/InstMaxIndex match jax.lax.top_k tie order).
"""

import os
import sys

import numpy as np

for _p in ("/opt/trn_rl_repo", "/opt/pypackages"):
    if _p not in sys.path and os.path.isdir(_p):
        sys.path.append(_p)

import concourse.bass as bass
import concourse.mybir as mybir
from concourse import bacc
from concourse.masks import make_identity
from concourse.tile import TileContext

F32 = mybir.dt.float32
F16 = mybir.dt.float16
I32 = mybir.dt.int32
U32 = mybir.dt.uint32

B, T, D, E, TOP_K = 8, 4096, 1024, 64, 2
N_CORES = 8
P = 128                    # SBUF partitions
D_CHUNKS = D // P          # 8 contraction chunks
TOK_BLK = 512              # tokens per block (matmul free dim)
SUBS = TOK_BLK // P        # 4 token tiles per block
SPLIT_SCALE = 2048.0       # 2^11 residual scale for the fp16 split


def _bcast(ap: bass.AP, n: int) -> bass.AP:
    """Append a step-0 dim of size n (free-dim broadcast for DVE reads)."""
    return bass.AP(tensor=ap.tensor, offset=ap.offset, ap=[*ap.ap, [0, n]])


def build_moe_router(t_core: int = T) -> bacc.Bacc:
    """Build the per-core Bass program. t_core tokens per core (mult of 512)."""
    assert t_core % TOK_BLK == 0
    n_blk = t_core // TOK_BLK
    n_tiles = t_core // P

    nc = bacc.Bacc("TRN2", target_bir_lowering=False, debug=False)

    xP = nc.dram_tensor("xP", [n_blk, P, 2, D_CHUNKS, TOK_BLK], F16, kind="ExternalInput")
    whT = nc.dram_tensor("whT", [D, E], F16, kind="ExternalInput")
    wlT = nc.dram_tensor("wlT", [D, E], F16, kind="ExternalInput")
    maskf = nc.dram_tensor("maskf", [P, n_tiles], F32, kind="ExternalInput")
    logits_d = nc.dram_tensor("logits", [n_blk, P, SUBS, E], F32, kind="ExternalOutput")
    probs_d = nc.dram_tensor("probs", [n_blk, P, SUBS, E], F32, kind="ExternalOutput")
    weights_d = nc.dram_tensor("weights", [P, n_tiles, TOP_K], F32, kind="ExternalOutput")
    indices_d = nc.dram_tensor("indices", [P, n_tiles, TOP_K], I32, kind="ExternalOutput")

    whT_t = whT.rearrange("(c p) e -> p c e", p=P)        # [128, 8, 64]
    wlT_t = wlT.rearrange("(c p) e -> p c e", p=P)

    MUL = mybir.AluOpType.mult
    ADD = mybir.AluOpType.add

    with TileContext(nc) as tc:
        with (
            tc.tile_pool(name="xpool", bufs=6) as xpool,
            tc.tile_pool(name="consts", bufs=1) as consts,
            tc.tile_pool(name="psT", bufs=3, space="PSUM") as psT,
            tc.tile_pool(name="psL", bufs=2, space="PSUM") as psL,
            tc.tile_pool(name="psink", bufs=1, space="PSUM") as psink,
            tc.tile_pool(name="stage", bufs=3) as stage,
            tc.tile_pool(name="small", bufs=6) as small,
            tc.tile_pool(name="accs", bufs=1) as accs,
        ):
            wh_sb = consts.tile([P, D_CHUNKS, E], F16)
            wl_sb = consts.tile([P, D_CHUNKS, E], F16)
            nc.sync.dma_start(out=wh_sb, in_=whT_t)
            nc.sync.dma_start(out=wl_sb, in_=wlT_t)
            maskf_sb = consts.tile([P, n_tiles], F32)
            nc.sync.dma_start(out=maskf_sb, in_=maskf[:, :])
            ident = consts.tile([E, E], F32)
            make_identity(nc, ident)

            top8 = accs.tile([P, n_tiles, 8], F32)
            idx8 = accs.tile([P, n_tiles, 8], U32)

            # HAM warmup: keep the PE busy ~5us starting right at kernel
            # entry (fed by a memset tile, no DMA wait) so real matmuls run
            # at 2.4 GHz instead of 1.2. Results discarded.
            warm_src = consts.tile([P, E], F16)
            nc.gpsimd.memset(warm_src, 0.0)
            warm_rhs = bass.AP(
                tensor=warm_src.tensor, offset=warm_src[:, 0:1].offset,
                ap=[warm_src[:, 0:1].ap[0], [0, TOK_BLK]],
            )
            warm_ps = psink.tile([E, TOK_BLK], F32)
            for w in range(12):
                nc.tensor.matmul(
                    warm_ps, lhsT=warm_src, rhs=warm_rhs,
                    start=(w == 0), stop=(w == 11), skip_group_check=True,
                )
            # ldweights absorb the W DMA-completion waits so real matmuls
            # carry at most one wait (walrus limit on Matmult sync waits).
            nc.tensor.ldweights(weights=wh_sb[:, 0, 0:1])
            nc.tensor.ldweights(weights=wl_sb[:, 0, 0:1])

            HC = D_CHUNKS // 2
            for blk in range(n_blk):
                x_sb = xpool.tile([P, 2, D_CHUNKS, TOK_BLK], F16)
                # two 1MB half-loads so matmuls start after the first half
                nc.sync.dma_start(out=x_sb[:, :, 0:HC, :], in_=xP[blk, :, :, 0:HC, :])
                nc.sync.dma_start(out=x_sb[:, :, HC:, :], in_=xP[blk, :, :, HC:, :])
                xh_sb = x_sb[:, 0]
                xl_sb = x_sb[:, 1]

                # logits.T: main term -> PSUM partitions 0:64, correction
                # terms (x2048) -> 64:128; the two column tiles run
                # concurrently on the PE array.
                lgT_ps = psT.tile([P, TOK_BLK], F32)
                for half in range(2):
                    # absorb this half's DMA wait on PE (ldweights is cheap)
                    nc.tensor.ldweights(weights=x_sb[:, 0, half * HC, 0:1])
                    for c in range(half * HC, (half + 1) * HC):
                        nc.tensor.matmul(
                            lgT_ps[0:E, :], lhsT=wh_sb[:, c, :], rhs=xh_sb[:, c, :],
                            start=(c == 0), stop=(c == D_CHUNKS - 1),
                            skip_group_check=True,
                        )
                        nc.tensor.matmul(
                            lgT_ps[E : 2 * E, :], lhsT=wl_sb[:, c, :], rhs=xh_sb[:, c, :],
                            start=(c == 0), stop=False, skip_group_check=True,
                        )
                        nc.tensor.matmul(
                            lgT_ps[E : 2 * E, :], lhsT=wh_sb[:, c, :], rhs=xl_sb[:, c, :],
                            start=False, stop=(c == D_CHUNKS - 1), skip_group_check=True,
                        )

                # lgT = correction/2048 + main   [64, 512] fp32 in SBUF
                # (DVE has one PSUM read port: ACT moves the main half to
                # SBUF, DVE adds the scaled correction from PSUM onto it.)
                lgT_sb = stage.tile([E, TOK_BLK], F32)
                nc.scalar.copy(lgT_sb, lgT_ps[0:E, :])
                nc.vector.scalar_tensor_tensor(
                    out=lgT_sb, in0=lgT_ps[E : 2 * E, :], scalar=1.0 / SPLIT_SCALE,
                    in1=lgT_sb, op0=MUL, op1=ADD,
                )

                # transpose back to [128 tok, 64 exp] tiles (PSUM, one bank)
                lg_ps = psL.tile([P, SUBS, E], F32)
                for sub in range(SUBS):
                    nc.tensor.matmul(
                        lg_ps[:, sub, :], lhsT=lgT_sb[:, sub * P : (sub + 1) * P],
                        rhs=ident, is_transpose=True, skip_group_check=True,
                    )

                mask_blk = maskf_sb[:, blk * SUBS : (blk + 1) * SUBS]  # [128, 4]

                # masked logits PSUM -> SBUF (also the DMA staging buffer)
                lg_sb = stage.tile([P, SUBS, E], F32)
                nc.vector.tensor_mul(lg_sb, lg_ps, _bcast(mask_blk, E))
                nc.scalar.dma_start(out=logits_d[blk, :, :, :], in_=lg_sb)

                # exps (unmasked is fine: masked rows are overridden later)
                exp_sb = stage.tile([P, SUBS, E], F32)
                nc.scalar.activation(
                    out=exp_sb, in_=lg_ps, func=mybir.ActivationFunctionType.Exp
                )

                sums = small.tile([P, SUBS, 1], F32)
                nc.vector.reduce_sum(sums, exp_sb, axis=mybir.AxisListType.X)
                r_t = small.tile([P, SUBS], F32)
                nc.vector.reciprocal(r_t, sums[:, :, 0])
                r2_t = small.tile([P, SUBS], F32)
                nc.vector.tensor_mul(r2_t, r_t, mask_blk)
                pr_sb = stage.tile([P, SUBS, E], F32)
                nc.gpsimd.tensor_mul(pr_sb, exp_sb, _bcast(r2_t[:, :], E))
                nc.scalar.dma_start(out=probs_d[blk, :, :, :], in_=pr_sb)

                for sub in range(SUBS):
                    i = blk * SUBS + sub
                    nc.vector.max(out=top8[:, i, :], in_=exp_sb[:, sub, :])
                    nc.vector.max_index(
                        out=idx8[:, i, :], in_max=top8[:, i, :],
                        in_values=exp_sb[:, sub, :],
                    )

            # ---- tail: renormalized top-2 weights + masked indices ----
            s_t = accs.tile([P, n_tiles], F32)
            nc.gpsimd.tensor_add(s_t, top8[:, :, 0], top8[:, :, 1])
            rs_t = accs.tile([P, n_tiles], F32)
            nc.vector.reciprocal(rs_t, s_t)
            nc.gpsimd.tensor_mul(rs_t, rs_t, maskf_sb)
            w_out = accs.tile([P, n_tiles, TOP_K], F32)
            for k in range(TOP_K):
                nc.gpsimd.tensor_mul(w_out[:, :, k], top8[:, :, k], rs_t)
            nc.scalar.dma_start(out=weights_d[:, :, :], in_=w_out)

            # indices: (idx + 1) * mask - 1  (exact in fp32)
            idxf = accs.tile([P, n_tiles, TOP_K], F32)
            nc.gpsimd.tensor_copy(idxf, idx8[:, :, 0:TOP_K])
            for k in range(TOP_K):
                nc.gpsimd.tensor_scalar_add(idxf[:, :, k], idxf[:, :, k], 1.0)
                nc.gpsimd.tensor_mul(idxf[:, :, k], idxf[:, :, k], maskf_sb)
                nc.gpsimd.tensor_scalar_add(idxf[:, :, k], idxf[:, :, k], -1.0)
            idxi = accs.tile([P, n_tiles, TOP_K], I32)
            nc.gpsimd.tensor_copy(idxi, idxf)
            nc.scalar.dma_start(out=indices_d[:, :, :], in_=idxi)

    # Legalization (splits >1-wait instructions into event-semaphore ops,
    # moves matmul waits to ldweights) — required by walrus codegen.
    nc.compile()
    return nc


_NC_CACHE: dict[int, bacc.Bacc] = {}


def _get_nc(t_core: int = T) -> bacc.Bacc:
    if t_core not in _NC_CACHE:
        _NC_CACHE[t_core] = build_moe_router(t_core)
    return _NC_CACHE[t_core]


def _split16(a: np.ndarray):
    hi = a.astype(np.float16)
    lo = ((a - hi.astype(np.float32)) * SPLIT_SCALE).astype(np.float16)
    return hi, lo


def _pack_x(xh: np.ndarray, xl: np.ndarray, t_core: int) -> np.ndarray:
    """2x [T, D] fp16 -> [n_blk, 128p, 2, 8c, 512t] matching the SBUF tiles."""
    n_blk = t_core // TOK_BLK
    both = np.stack([xh, xl], axis=0)  # [2, T, D]
    return np.ascontiguousarray(
        both.reshape(2, n_blk, TOK_BLK, D_CHUNKS, P).transpose(1, 4, 0, 3, 2)
    )


def make_in_maps(x: np.ndarray, x_mask: np.ndarray, W: np.ndarray):
    """Shard full inputs into per-core input maps (host-side layout prep)."""
    t_core = x.shape[1]
    n_tiles = t_core // P
    wh, wl = _split16(np.asarray(W, dtype=np.float32))
    whT = np.ascontiguousarray(wh.T)
    wlT = np.ascontiguousarray(wl.T)
    in_maps = []
    for b in range(x.shape[0]):
        xh, xl = _split16(np.asarray(x[b], dtype=np.float32))
        mf = np.ascontiguousarray(
            np.asarray(x_mask[b], dtype=np.float32).reshape(n_tiles, P).T
        )
        in_maps.append(
            {
                "xP": _pack_x(xh, xl, t_core),
                "whT": whT,
                "wlT": wlT,
                "maskf": mf,
            }
        )
    return in_maps


def _unpack_te(a: np.ndarray, t_core: int) -> np.ndarray:
    """[n_blk, 128p, 4sub, E] -> [T, E]."""
    return np.ascontiguousarray(
        a.transpose(0, 2, 1, 3).reshape(t_core, a.shape[-1])
    )


def _unpack_tk(a: np.ndarray, t_core: int) -> np.ndarray:
    """[128p, n_tiles, K] -> [T, K]."""
    return np.ascontiguousarray(a.transpose(1, 0, 2).reshape(t_core, a.shape[-1]))


def run_kernel(x, x_mask, W, trace: bool = False, trace_kwargs: dict | None = None):
    """Run on hardware; returns (outputs_tuple, BassKernelResults)."""
    from concourse.bass_utils import run_bass_kernel_spmd

    x = np.asarray(x)
    x_mask = np.asarray(x_mask)
    W = np.asarray(W)
    n_cores, t_core = x.shape[0], x.shape[1]
    nc = _get_nc(t_core)
    in_maps = make_in_maps(x, x_mask, W)
    res = run_bass_kernel_spmd(
        nc,
        in_maps,
        core_ids=list(range(n_cores)),
        trace=trace,
        **(trace_kwargs or {}),
    )
    ew = np.stack([_unpack_tk(res.results[b]["weights"], t_core) for b in range(n_cores)])
    ei = np.stack([_unpack_tk(res.results[b]["indices"], t_core) for b in range(n_cores)])
    rl = np.stack([_unpack_te(res.results[b]["logits"], t_core) for b in range(n_cores)])
    rp = np.stack([_unpack_te(res.results[b]["probs"], t_core) for b in range(n_cores)])
    return (ew, ei, rl, rp), res


def kernel(**inputs):
    outs, _ = run_kernel(
        inputs["x"], inputs["x_mask"], inputs["W"],
        trace=os.environ.get("MOE_TRACE", "") == "1",
    )
    return outs


# revision 19
# speedup vs baseline: 1.1199x; 1.0849x over previous
"""MoE router kernel for Trainium2 (Bass/Tile), SPMD across 8 NeuronCores.

Problem: nn_MoERouter (B=8, T=4096, D=1024, E=64, TOP_K=2).

  router_logits = (x * mask) @ W.T * mask          # (B, T, E)
  router_probs  = softmax(router_logits) * mask
  expert_weights, expert_indices = top_k(probs, 2), renormalized, masked
  padded tokens get index -1

Sharding: data-parallel over the batch axis; core b handles x[b] (4096
tokens). W is tiny and replicated. No cross-core communication.

Matmul strategy (per core): plain fp32 matmul on TRN2 lowers to 2
half-rate passes (~8 ns/output-col measured) and float32r is only
~tf32-accurate (1.5e-4 — would flip near-tie expert indices). Instead we
use an error-compensated fp16 split computed on the host:

  x = xh + xls/2048,  W = Wh + Wls/2048   (xh/xls/Wh/Wls all fp16)
  logits = xh@Wh + (xh@Wls + xls@Wh)/2048    (drops xls@Wls ~ 2^-22)

Max logits error ~4e-6 (verified on the real inputs: 0/65536 index
flips), DMA volume unchanged (2+2 bytes/elem), and the matmuls run at
full 1 cycle/row rate.

Layouts: the contraction (d) must be on SBUF partitions and DMA
descriptors want long contiguous per-partition runs, so the host
pre-tiles x into the exact per-block SBUF layout
[n_blk, 128p, 8chunk, 512tok] (every DMA is a fully contiguous 1 MB
read, 8 KB per partition). logits/probs are likewise written in packed
per-block layout [n_blk, 128p, 4sub, 64e] (1 KB/partition runs) and
unpacked on the host.

Compute: W-stationary matmuls, N=512 tokens, out = logits.T [64, 512]
in PSUM. The main term accumulates in PSUM partitions 0:64 and both
correction terms in 64:128 (concurrent column-packed PE tiles), then
ACT moves the main half to SBUF and one DVE scalar_tensor_tensor adds
correction/2048 (DVE has a single PSUM read port). Four PE transposes
per block bring logits back to [128 tok, 64 exp], ACT does a batched
exp, DVE does softmax scaling + top-8 with indices
(InstMax/InstMaxIndex match jax.lax.top_k tie order).
"""

import os
import sys

import numpy as np

for _p in ("/opt/trn_rl_repo", "/opt/pypackages"):
    if _p not in sys.path and os.path.isdir(_p):
        sys.path.append(_p)

import concourse.bass as bass
import concourse.mybir as mybir
from concourse import bacc
from concourse.masks import make_identity
from concourse.tile import TileContext

F32 = mybir.dt.float32
F16 = mybir.dt.float16
I32 = mybir.dt.int32
U32 = mybir.dt.uint32

B, T, D, E, TOP_K = 8, 4096, 1024, 64, 2
N_CORES = 8
P = 128                    # SBUF partitions
D_CHUNKS = D // P          # 8 contraction chunks
TOK_BLK = 512              # tokens per block (matmul free dim)
SUBS = TOK_BLK // P        # 4 token tiles per block
SPLIT_SCALE = 2048.0       # 2^11 residual scale for the fp16 split


def _bcast(ap: bass.AP, n: int) -> bass.AP:
    """Append a step-0 dim of size n (free-dim broadcast for DVE reads)."""
    return bass.AP(tensor=ap.tensor, offset=ap.offset, ap=[*ap.ap, [0, n]])


def build_moe_router(t_core: int = T) -> bacc.Bacc:
    """Build the per-core Bass program. t_core tokens per core (mult of 512)."""
    assert t_core % TOK_BLK == 0
    n_blk = t_core // TOK_BLK
    n_tiles = t_core // P

    nc = bacc.Bacc("TRN2", target_bir_lowering=False, debug=False)

    xP = nc.dram_tensor("xP", [n_blk, P, 2, D_CHUNKS, TOK_BLK], F16, kind="ExternalInput")
    whP = nc.dram_tensor("whP", [P, D_CHUNKS, E], F16, kind="ExternalInput")
    wlP = nc.dram_tensor("wlP", [P, D_CHUNKS, E], F16, kind="ExternalInput")
    maskf = nc.dram_tensor("maskf", [P, n_tiles], F32, kind="ExternalInput")
    logits_d = nc.dram_tensor("logits", [n_blk, P, SUBS, E], F32, kind="ExternalOutput")
    probs_d = nc.dram_tensor("probs", [n_blk, P, SUBS, E], F32, kind="ExternalOutput")
    weights_d = nc.dram_tensor("weights", [P, n_tiles, TOP_K], F32, kind="ExternalOutput")
    indices_d = nc.dram_tensor("indices", [P, n_tiles, TOP_K], I32, kind="ExternalOutput")

    MUL = mybir.AluOpType.mult
    ADD = mybir.AluOpType.add

    with TileContext(nc) as tc:
        with (
            tc.tile_pool(name="xpool", bufs=6) as xpool,
            tc.tile_pool(name="consts", bufs=1) as consts,
            tc.tile_pool(name="psT", bufs=4, space="PSUM") as psT,
            tc.tile_pool(name="psL", bufs=2, space="PSUM") as psL,
            tc.tile_pool(name="psink", bufs=1, space="PSUM") as psink,
            tc.tile_pool(name="stage", bufs=4) as stage,
            tc.tile_pool(name="small", bufs=6) as small,
            tc.tile_pool(name="accs", bufs=1) as accs,
        ):
            wh_sb = consts.tile([P, D_CHUNKS, E], F16)
            wl_sb = consts.tile([P, D_CHUNKS, E], F16)
            nc.scalar.dma_start(out=wh_sb, in_=whP[:, :, :])
            nc.scalar.dma_start(out=wl_sb, in_=wlP[:, :, :])
            maskf_sb = consts.tile([P, n_tiles], F32)
            nc.scalar.dma_start(out=maskf_sb, in_=maskf[:, :])
            ident = consts.tile([E, E], F32)
            make_identity(nc, ident)

            top8 = accs.tile([P, n_tiles, 8], F32)
            idx8 = accs.tile([P, n_tiles, 8], U32)
            w_out = accs.tile([P, n_tiles, TOP_K], F32)
            idxi = accs.tile([P, n_tiles, TOP_K], I32)

            # HAM warmup: keep the PE busy ~5us starting right at kernel
            # entry (fed by a memset tile, no DMA wait) so real matmuls run
            # at 2.4 GHz instead of 1.2. Results discarded.
            warm_src = consts.tile([P, E], F16)
            nc.gpsimd.memset(warm_src, 0.0)
            warm_rhs = bass.AP(
                tensor=warm_src.tensor, offset=warm_src[:, 0:1].offset,
                ap=[warm_src[:, 0:1].ap[0], [0, TOK_BLK]],
            )
            warm_ps = psink.tile([E, TOK_BLK], F32)
            for w in range(14):
                nc.tensor.matmul(
                    warm_ps, lhsT=warm_src, rhs=warm_rhs,
                    start=(w == 0), stop=(w == 13), skip_group_check=True,
                )
            # ldweights absorb the W DMA-completion waits so real matmuls
            # carry at most one wait (walrus limit on Matmult sync waits).
            nc.tensor.ldweights(weights=wh_sb[:, 0, 0:1])
            nc.tensor.ldweights(weights=wl_sb[:, 0, 0:1])

            HC = D_CHUNKS // 2
            for blk in range(n_blk):
                x_sb = xpool.tile([P, 2, D_CHUNKS, TOK_BLK], F16)
                # two 1MB half-loads so matmuls start after the first half
                nc.sync.dma_start(out=x_sb[:, :, 0:HC, :], in_=xP[blk, :, :, 0:HC, :])
                nc.sync.dma_start(out=x_sb[:, :, HC:, :], in_=xP[blk, :, :, HC:, :])
                xh_sb = x_sb[:, 0]
                xl_sb = x_sb[:, 1]

                # logits.T: main term -> PSUM partitions 0:64, correction
                # terms (x2048) -> 64:128; the two column tiles run
                # concurrently on the PE array.
                lgT_ps = psT.tile([P, TOK_BLK], F32)
                for half in range(2):
                    # absorb this half's DMA wait on PE (ldweights is cheap)
                    nc.tensor.ldweights(weights=x_sb[:, 0, half * HC, 0:1])
                    for c in range(half * HC, (half + 1) * HC):
                        nc.tensor.matmul(
                            lgT_ps[0:E, :], lhsT=wh_sb[:, c, :], rhs=xh_sb[:, c, :],
                            start=(c == 0), stop=(c == D_CHUNKS - 1),
                            skip_group_check=True,
                        )
                        nc.tensor.matmul(
                            lgT_ps[E : 2 * E, :], lhsT=wl_sb[:, c, :], rhs=xh_sb[:, c, :],
                            start=(c == 0), stop=False, skip_group_check=True,
                        )
                        nc.tensor.matmul(
                            lgT_ps[E : 2 * E, :], lhsT=wh_sb[:, c, :], rhs=xl_sb[:, c, :],
                            start=False, stop=(c == D_CHUNKS - 1), skip_group_check=True,
                        )

                # lgT = correction/2048 + main   [64, 512] fp32 in SBUF
                # (DVE has one PSUM read port: ACT moves the main half to
                # SBUF, DVE adds the scaled correction from PSUM onto it.)
                lgT_sb = stage.tile([E, TOK_BLK], F32)
                nc.scalar.copy(lgT_sb, lgT_ps[0:E, :])
                nc.vector.scalar_tensor_tensor(
                    out=lgT_sb, in0=lgT_ps[E : 2 * E, :], scalar=1.0 / SPLIT_SCALE,
                    in1=lgT_sb, op0=MUL, op1=ADD,
                )

                # transpose back to [128 tok, 64 exp] tiles (PSUM, one bank)
                lg_ps = psL.tile([P, SUBS, E], F32)
                for sub in range(SUBS):
                    nc.tensor.matmul(
                        lg_ps[:, sub, :], lhsT=lgT_sb[:, sub * P : (sub + 1) * P],
                        rhs=ident, is_transpose=True, skip_group_check=True,
                    )

                mask_blk = maskf_sb[:, blk * SUBS : (blk + 1) * SUBS]  # [128, 4]

                # masked logits PSUM -> SBUF (also the DMA staging buffer)
                lg_sb = stage.tile([P, SUBS, E], F32)
                nc.vector.tensor_mul(lg_sb, lg_ps, _bcast(mask_blk, E))
                nc.scalar.dma_start(out=logits_d[blk, :, :, :], in_=lg_sb)

                # exps (unmasked is fine: masked rows are overridden later)
                exp_sb = stage.tile([P, SUBS, E], F32)
                nc.scalar.activation(
                    out=exp_sb, in_=lg_ps, func=mybir.ActivationFunctionType.Exp
                )
                sums = small.tile([P, SUBS, 1], F32)
                nc.vector.reduce_sum(sums, exp_sb, axis=mybir.AxisListType.X)
                r2_t = small.tile([P, SUBS], F32)
                nc.vector.reciprocal(r2_t, sums[:, :, 0])
                nc.vector.tensor_mul(r2_t, r2_t, mask_blk)
                pr_sb = stage.tile([P, SUBS, E], F32)
                nc.gpsimd.tensor_mul(pr_sb, exp_sb, _bcast(r2_t[:, :], E))
                nc.scalar.dma_start(out=probs_d[blk, :, :, :], in_=pr_sb)

                for sub in range(SUBS):
                    i = blk * SUBS + sub
                    nc.vector.max(out=top8[:, i, :], in_=exp_sb[:, sub, :])
                    nc.vector.max_index(
                        out=idx8[:, i, :], in_max=top8[:, i, :],
                        in_values=exp_sb[:, sub, :],
                    )

                # per-block renormalized top-2 weights (DVE, small fast ops)
                # + masked indices (gpsimd); keeps the end-of-kernel tail to
                # one block's worth of work.
                bsl = slice(blk * SUBS, (blk + 1) * SUBS)
                s4 = small.tile([P, SUBS], F32)
                nc.vector.tensor_add(s4, top8[:, bsl, 0], top8[:, bsl, 1])
                rs4 = small.tile([P, SUBS], F32)
                nc.vector.reciprocal(rs4, s4)
                nc.vector.tensor_mul(rs4, rs4, mask_blk)
                for k in range(TOP_K):
                    nc.vector.tensor_mul(w_out[:, bsl, k], top8[:, bsl, k], rs4)
                idxf4 = small.tile([P, SUBS, TOP_K], F32)
                nc.gpsimd.tensor_copy(idxf4, idx8[:, bsl, 0:TOP_K])
                for k in range(TOP_K):
                    nc.gpsimd.tensor_scalar_add(idxf4[:, :, k], idxf4[:, :, k], 1.0)
                    nc.gpsimd.tensor_mul(idxf4[:, :, k], idxf4[:, :, k], mask_blk)
                    nc.gpsimd.tensor_scalar_add(idxf4[:, :, k], idxf4[:, :, k], -1.0)
                nc.gpsimd.tensor_copy(idxi[:, bsl, :], idxf4)

            nc.scalar.dma_start(out=weights_d[:, :, :], in_=w_out)
            nc.scalar.dma_start(out=indices_d[:, :, :], in_=idxi)

    # Legalization (splits >1-wait instructions into event-semaphore ops,
    # moves matmul waits to ldweights) — required by walrus codegen.
    nc.compile()
    return nc


_NC_CACHE: dict[int, bacc.Bacc] = {}


def _get_nc(t_core: int = T) -> bacc.Bacc:
    if t_core not in _NC_CACHE:
        _NC_CACHE[t_core] = build_moe_router(t_core)
    return _NC_CACHE[t_core]


def _split16(a: np.ndarray):
    hi = a.astype(np.float16)
    lo = ((a - hi.astype(np.float32)) * SPLIT_SCALE).astype(np.float16)
    return hi, lo


def _pack_x(xh: np.ndarray, xl: np.ndarray, t_core: int) -> np.ndarray:
    """2x [T, D] fp16 -> [n_blk, 128p, 2, 8c, 512t] matching the SBUF tiles."""
    n_blk = t_core // TOK_BLK
    both = np.stack([xh, xl], axis=0)  # [2, T, D]
    return np.ascontiguousarray(
        both.reshape(2, n_blk, TOK_BLK, D_CHUNKS, P).transpose(1, 4, 0, 3, 2)
    )


def make_in_maps(x: np.ndarray, x_mask: np.ndarray, W: np.ndarray):
    """Shard full inputs into per-core input maps (host-side layout prep)."""
    t_core = x.shape[1]
    n_tiles = t_core // P
    wh, wl = _split16(np.asarray(W, dtype=np.float32))
    # [E, D] -> [128p, 8c, E] matching the SBUF tile (dense 1KB runs)
    whP = np.ascontiguousarray(wh.T.reshape(D_CHUNKS, P, E).transpose(1, 0, 2))
    wlP = np.ascontiguousarray(wl.T.reshape(D_CHUNKS, P, E).transpose(1, 0, 2))
    in_maps = []
    for b in range(x.shape[0]):
        xh, xl = _split16(np.asarray(x[b], dtype=np.float32))
        mf = np.ascontiguousarray(
            np.asarray(x_mask[b], dtype=np.float32).reshape(n_tiles, P).T
        )
        in_maps.append(
            {
                "xP": _pack_x(xh, xl, t_core),
                "whP": whP,
                "wlP": wlP,
                "maskf": mf,
            }
        )
    return in_maps


def _unpack_te(a: np.ndarray, t_core: int) -> np.ndarray:
    """[n_blk, 128p, 4sub, E] -> [T, E]."""
    return np.ascontiguousarray(
        a.transpose(0, 2, 1, 3).reshape(t_core, a.shape[-1])
    )


def _unpack_tk(a: np.ndarray, t_core: int) -> np.ndarray:
    """[128p, n_tiles, K] -> [T, K]."""
    return np.ascontiguousarray(a.transpose(1, 0, 2).reshape(t_core, a.shape[-1]))


def run_kernel(x, x_mask, W, trace: bool = False, trace_kwargs: dict | None = None):
    """Run on hardware; returns (outputs_tuple, BassKernelResults)."""
    from concourse.bass_utils import run_bass_kernel_spmd

    x = np.asarray(x)
    x_mask = np.asarray(x_mask)
    W = np.asarray(W)
    n_cores, t_core = x.shape[0], x.shape[1]
    nc = _get_nc(t_core)
    in_maps = make_in_maps(x, x_mask, W)
    res = run_bass_kernel_spmd(
        nc,
        in_maps,
        core_ids=list(range(n_cores)),
        trace=trace,
        **(trace_kwargs or {}),
    )
    ew = np.stack([_unpack_tk(res.results[b]["weights"], t_core) for b in range(n_cores)])
    ei = np.stack([_unpack_tk(res.results[b]["indices"], t_core) for b in range(n_cores)])
    rl = np.stack([_unpack_te(res.results[b]["logits"], t_core) for b in range(n_cores)])
    rp = np.stack([_unpack_te(res.results[b]["probs"], t_core) for b in range(n_cores)])
    return (ew, ei, rl, rp), res


def kernel(**inputs):
    outs, _ = run_kernel(
        inputs["x"], inputs["x_mask"], inputs["W"],
        trace=os.environ.get("MOE_TRACE", "") == "1",
    )
    return outs
